# revision 24
# baseline (speedup 1.0000x reference)
"""Trainium2 Bass kernel for nn_DirectionalProcessor — flipped-operand variant.

Same folded-conv math as v3:
    M_d = Wd[d] @ Wc[:, d*C:(d+1)*C].T          (C x C)
    out[p] = sum_d x[p - (dy_d, dx_d)] @ M_d + bc
but with the PE operands flipped: the stationary operand is an M_d chunk
[128 c-part, 128 o-part] and the MOVING operand is the x window, streamed
512 positions per matmul. Wins vs v3:
  - half the matmuls (N=512 vs 256) -> half the NX dispatch overhead
  - 65-stride spatial packing (single shared pad column between rows
    instead of two) -> 4160 padded positions/image instead of 4224, and a
    fractional tail block costs only its 64 columns (positions are the
    moving dim), total ~1.5% less PE streaming
Output lands channel-major [oc, pos]; the host transposes it back (free).

Sharding: data-parallel over batch, 2 images/core, weights replicated,
no collectives. Host folds M (fp32), pre-casts x to fp16, upcasts the
fp16 output. Startup: single SP-ring priority stream (head x strip,
8 per-direction M pieces, bias, rest); tap-major head block so the PE
tracks the M pieces as they land; warm-up matmuls bridge the HAM window.
"""

import numpy as np

import concourse.bass as bass
import concourse.bacc as bacc
import concourse.mybir as mybir
import concourse.tile as tile
from concourse.bass_utils import run_bass_kernel_spmd

B, H, W, C = 16, 64, 64, 256
DIRECTIONS = [(0, -1), (1, -1), (1, 0), (1, 1), (0, 1), (-1, 1), (-1, 0), (-1, -1)]
N_CORES = 8
BPC = B // N_CORES  # images per core
SP65 = W + 1  # 65: row stride, single shared pad column
NQ4 = H * SP65  # 4160 padded output positions per image
PADL = SP65 + 1  # 66 zeros before/after the body (max |delta| = 66)
XF4 = PADL + NQ4 + PADL  # 4292
F16 = mybir.dt.float16
F32 = mybir.dt.float32

N_WARM = 32
BLOCKS = [(b * 512, 512) for b in range(8)] + [(4096, 64)]  # (pos0, n)

LAST_RESULTS = None


def build_bass() -> bass.Bass:
    nc = bacc.Bacc(None)

    xp_d = nc.dram_tensor("xp", [BPC, C, XF4], F16, kind="ExternalInput")
    # folded M: [d, p=c%128, cc, oc, o2] -> 8 per-direction 128KB pieces,
    # interleaved with the head x strips on the ring for tap-major pacing
    m_d = nc.dram_tensor("m", [8, 128, 2, 2, 128], F16, kind="ExternalInput")
    # bias broadcast tiles, one per out-channel chunk: [oc, p, 512]
    b_d = nc.dram_tensor("bias", [2, 128, 512], F32, kind="ExternalInput")
    out_d = nc.dram_tensor("out", [BPC, 2, 128, NQ4], F16, kind="ExternalOutput")

    with tile.TileContext(nc) as tc:
        with (
            tc.tile_pool(name="const", bufs=1) as const,
            tc.tile_pool(name="psum", bufs=7, space="PSUM") as psum_pool,
            tc.tile_pool(name="warmps", bufs=1, space="PSUM") as warm_pool,
            tc.tile_pool(name="osb", bufs=4) as osb_pool,
        ):
            # ---- single SP-ring input stream in priority order ----
            m16 = const.tile([128, 8, 2, 2, 128], F16, tag="m16")
            xts = [
                [
                    const.tile(
                        [128, XF4], F16, tag=f"xp_{img}_{ch}", name=f"xp_{img}_{ch}"
                    )
                    for ch in range(2)
                ]
                for img in range(BPC)
            ]
            bias_sb = const.tile([128, 2, 512], F32, tag="bias_sb")

            def xdma(img, ch, lo, hi):
                nc.sync.dma_start(
                    out=xts[img][ch][:, lo:hi],
                    in_=xp_d[:][img, ch * 128 : (ch + 1) * 128, lo:hi],
                )

            SA, SB0, SB1 = 644, 1156, 1952
            for ch in range(2):
                xdma(0, ch, 0, SA)
            for dd in range(8):
                nc.sync.dma_start(out=m16[:, dd], in_=m_d[:][dd])
            for ch in range(2):
                xdma(0, ch, SA, SB0)
            for oc in range(2):
                nc.sync.dma_start(out=bias_sb[:, oc], in_=b_d[:][oc])
            for ch in range(2):
                xdma(0, ch, SB0, SB1)
            for ch in range(2):
                xdma(0, ch, SB1, XF4)
            for ch in range(2):
                xdma(1, ch, 0, XF4 // 2)
            for ch in range(2):
                xdma(1, ch, XF4 // 2, XF4)

            # ---- PE pre-warm ----
            warm16 = const.tile([128, 128], F16, tag="warm16")
            nc.vector.memset(warm16[:], 0.0)
            wps = warm_pool.tile([128, 512], F32, tag="warm")
            for _ in range(N_WARM):
                nc.tensor.matmul(wps[:, 0:128], lhsT=warm16[:], rhs=warm16[:])

            # ---- main conv loop: stationary = M chunk, moving = positions --
            deltas = [-(dy * SP65 + dx) for (dx, dy) in DIRECTIONS]

            def evac_store(img, bi, pos0, n, oc, pt):
                ot = osb_pool.tile(
                    [128, 512], F16, tag="osb", name=f"ot{img}_{bi}_{oc}"
                )
                nc.vector.tensor_add(ot[:, :n], pt[:, :n], bias_sb[:, oc, :n])
                nc.scalar.dma_start(
                    out=out_d[:][img, oc, :, pos0 : pos0 + n], in_=ot[:, :n]
                )

            # head: block 0 of image 0 tap-major, DMA-paced
            hpts = [
                psum_pool.tile([128, 512], F32, tag="ps", name=f"psh_{oc}")
                for oc in range(2)
            ]
            for di in range(8):
                s = PADL + deltas[di]
                for oc in range(2):
                    for ch in range(2):
                        nc.tensor.matmul(
                            hpts[oc][:],
                            lhsT=m16[:, di, ch, oc],
                            rhs=xts[0][ch][:, s : s + 512],
                            start=(di == 0 and ch == 0),
                            stop=(di == 7 and ch == 1),
                        )
            for oc in range(2):
                evac_store(0, 0, 0, 512, oc, hpts[oc])

            for img in range(BPC):
                order = BLOCKS if img == 0 else [BLOCKS[-1]] + BLOCKS[:-1]
                for bi, (pos0, n) in enumerate(order):
                    if img == 0 and bi == 0:
                        continue
                    for oc in range(2):
                        pt = psum_pool.tile(
                            [128, 512], F32, tag="ps", name=f"ps{img}_{bi}_{oc}"
                        )
                        for di in range(8):
                            s = PADL + pos0 + deltas[di]
                            for ch in range(2):
                                nc.tensor.matmul(
                                    pt[:, :n],
                                    lhsT=m16[:, di, ch, oc],
                                    rhs=xts[img][ch][:, s : s + n],
                                    start=(di == 0 and ch == 0),
                                    stop=(di == 7 and ch == 1),
                                )
                        evac_store(img, bi, pos0, n, oc, pt)

    nc.finalize()
    return nc


def _host_prep(grid_embedding, Wd, Wc, bc):
    g = np.asarray(grid_embedding, dtype=np.float32)
    gpad = np.zeros((B, C, XF4), np.float16)
    body = gpad[:, :, PADL : PADL + NQ4].reshape(B, C, H, SP65)
    body[:, :, :, :W] = g.transpose(0, 3, 1, 2)
    # fold: M[d, c, o] = sum_e Wd[d, c, e] * Wc[o, d*C + e]  (fp32 accumulate)
    wcr = np.asarray(Wc, np.float32).reshape(C, 8, C)  # [o, d, e]
    m = np.einsum("dce,ode->dco", np.asarray(Wd, np.float32), wcr)
    # -> [d, p=c%128, cc, oc, o2] fp16
    m16 = np.ascontiguousarray(
        m.reshape(8, 2, 128, 2, 128).transpose(0, 2, 1, 3, 4).astype(np.float16)
    )
    bias = np.ascontiguousarray(
        np.broadcast_to(
            np.asarray(bc, np.float32).reshape(2, 128)[:, :, None], (2, 128, 512)
        )
    )
    return gpad, m16, bias


def make_in_maps(gpad, m16, bias):
    return [
        {
            "xp": np.ascontiguousarray(gpad[core * BPC : (core + 1) * BPC]),
            "m": m16,
            "bias": bias,
        }
        for core in range(N_CORES)
    ]


def _unpad_out(out4):
    # [BPC, 2, 128, NQ4] -> [BPC, H, W, C]
    o = out4.astype(np.float32).reshape(BPC, C, H, SP65)
    return o[:, :, :, :W].transpose(0, 2, 3, 1)


_NC_CACHE = {}


def kernel(grid_embedding, Wd, Wc, bc):
    global LAST_RESULTS
    gpad, m16, bias = _host_prep(grid_embedding, Wd, Wc, bc)

    if "nc" not in _NC_CACHE:
        _NC_CACHE["nc"] = build_bass()
    nc = _NC_CACHE["nc"]

    in_maps = make_in_maps(gpad, m16, bias)
    res = run_bass_kernel_spmd(nc, in_maps, core_ids=list(range(N_CORES)))
    LAST_RESULTS = res
    out = np.concatenate([_unpad_out(r["out"]) for r in res.results], axis=0)
    return np.ascontiguousarray(out.reshape(B, H, W, C))


if __name__ == "__main__":
    rng = np.random.default_rng(0)
    inputs = {
        "grid_embedding": rng.standard_normal((B, H, W, C), dtype=np.float32),
        "Wd": (rng.standard_normal((8, C, C)) * 0.01).astype(np.float32),
        "Wc": (rng.standard_normal((C, 8 * C)) * 0.02).astype(np.float32),
        "bc": (rng.standard_normal(C) * 0.02).astype(np.float32),
    }
    out = kernel(**inputs)
    print("out", out.shape, out.dtype)
